# revision 19
# baseline (speedup 1.0000x reference)
"""Autoformer forward pass on Trainium2 (Bass/Tile), data-parallel over batch (8 cores).

- One NeuronCore per batch element (B=8), same NEFF on all cores (SPMD).
- Residual stream in NORMAL layout [t, c] (t on partitions, tile-major), PE transposes
  to [c, t] for channel-contraction matmuls.
- Autocorrelation mean-corr via Gram matrix with swapped roles:
  corrswap[d] = sum_t <q_t, k_(t+d) mod L> / 512  (= corr[(L-d) mod L]),
  diagonal (skew) reduction via DRAM round-trip with stride-(W+1) AP.
- top-k via threshold: tau = k-th largest (3 rounds of max8 + match_replace), softmax
  weights = exp(corr - max) * (corr >= tau) / Z. Filter applied as circulant matmul whose
  stationary tiles come from a DRAM "staircase"-replicated filter row.
- series_decomp as banded matmul with host-built (I-A)^T.
- Only last output timestep needed -> decoder trend path reduced to 3 rows.
"""

import math
import numpy as np
import ml_dtypes

import bass_rust
import concourse.bacc as bacc
import concourse.mybir as mybir
from concourse.tile import TileContext, add_dep_helper
from concourse.bass_utils import run_bass_kernel_spmd

F32 = mybir.dt.float32
BF16 = mybir.dt.bfloat16
AF = mybir.ActivationFunctionType
ALU = mybir.AluOpType
AXX = mybir.AxisListType.X

OUT_V = 21
D = 512
DFF = 2048
LEN = 1023
LDEC = 513
MA = 25
BATCH = 8
NEG = -3.0e38
GW = 1536              # gram scratch row width
PITCH = 8192           # staircase pitch


def tiles_of(L):
    out, t0 = [], 0
    while t0 < L:
        out.append((t0, min(128, L - t0)))
        t0 += 128
    return out


def rap(t, offset, dims):
    return bass_rust.AP(t, offset, [list(d) for d in dims])


def moving_avg_matrix(L, k=MA):
    A = np.zeros((L, L), np.float64)
    pad = (k - 1) // 2
    for t in range(L):
        for d in range(-pad, pad + 1):
            A[t, min(max(t + d, 0), L - 1)] += 1.0 / k
    return A.astype(np.float32)


def host_consts(params):
    c = {}
    c["ident"] = np.eye(128, dtype=np.float32)
    c["ones_col"] = np.ones((128, 1), np.float32)
    c["inv512_col"] = np.full((128, 1), 1.0 / 512.0, np.float32)
    c["ones_row"] = np.ones((1, 128), np.float32)

    for L, tag in ((LEN, "e"), (LDEC, "d")):
        A = moving_avg_matrix(L)
        IA = np.eye(L, dtype=np.float32) - A
        nt = (L + 127) // 128
        IAT = np.zeros((nt * 128, nt * 128), np.float32)
        IAT[:L, :L] = IA.T
        c[f"iat_{tag}"] = IAT
        if tag == "d":
            RT = np.zeros((nt * 128, 3), np.float32)
            RT[:L, :] = A[[0, L - 2, L - 1], :].T
            c["a_rows_d"] = RT

    A = moving_avg_matrix(LEN)
    IA = np.eye(LEN, dtype=np.float32) - A
    BT = np.zeros((8 * 128, 512), np.float32)
    BT[:LEN, :] = IA[511:1023, :].T
    c["sinit_bt"] = BT

    def convw(w):
        w = np.asarray(w)
        return [np.ascontiguousarray(w[:, :, i].T, np.float32) for i in range(3)]

    for i, wd in enumerate(convw(params["enc_emb"])):
        c[f"enc_emb_{i}"] = wd
    for i, wd in enumerate(convw(params["dec_emb"])):
        c[f"dec_emb_{i}"] = wd

    pos = np.arange(LDEC)[:, None].astype(np.float32)
    div = np.exp(np.arange(0, D, 2).astype(np.float32) * -(math.log(10000.0) / D))
    pe = np.zeros((LDEC, D), np.float32)
    pe[:, 0::2] = np.sin(pos * div)
    pe[:, 1::2] = np.cos(pos * div)
    c["pos_emb"] = pe

    def ac_consts(pref, p, pre):
        wq, bq = np.asarray(p[pre + "wq"]), np.asarray(p[pre + "bq"])
        wk, bk = np.asarray(p[pre + "wk"]), np.asarray(p[pre + "bk"])
        wv, bv = np.asarray(p[pre + "wv"]), np.asarray(p[pre + "bv"])
        wo, bo = np.asarray(p[pre + "wo"]), np.asarray(p[pre + "bo"])
        c[f"{pref}_wqT"] = np.ascontiguousarray(wq.T, np.float32)
        c[f"{pref}_wkT"] = np.ascontiguousarray(wk.T, np.float32)
        c[f"{pref}_wvoT"] = np.ascontiguousarray((wo @ wv).T, np.float32)
        c[f"{pref}_bq"] = np.ascontiguousarray(bq.reshape(4, 128).T, np.float32)
        c[f"{pref}_bk"] = np.ascontiguousarray(bk.reshape(4, 128).T, np.float32)
        c[f"{pref}_bvo"] = (wo @ bv + bo).reshape(1, D).astype(np.float32)

    for li, p in enumerate(params["enc_layers"]):
        ac_consts(f"enc{li}", p, "")
        c[f"enc{li}_conv1T"] = np.ascontiguousarray(np.asarray(p["conv1"]).T, np.float32)
        c[f"enc{li}_conv2R"] = np.ascontiguousarray(np.asarray(p["conv2"]).T).astype(ml_dtypes.bfloat16)
    dp = params["dec_layers"][0]
    ac_consts("dec_sa", dp, "sa_")
    ac_consts("dec_ca", dp, "ca_")
    c["dec_conv1T"] = np.ascontiguousarray(np.asarray(dp["conv1"]).T, np.float32)
    c["dec_conv2R"] = np.ascontiguousarray(np.asarray(dp["conv2"]).T).astype(ml_dtypes.bfloat16)
    tw = np.asarray(dp["trend_proj"])
    for i in range(3):
        c[f"trend_w{i}"] = np.ascontiguousarray(tw[:, :, i].T, np.float32)

    c["proj_wT"] = np.ascontiguousarray(np.asarray(params["proj_w"]).T, np.float32)
    c["proj_b"] = np.asarray(params["proj_b"]).reshape(1, OUT_V).astype(np.float32)
    return c


def build_program(dbg=()):
    nc = bacc.Bacc("TRN2", target_bir_lowering=False, debug=False)
    inputs = {}

    def inp(name, shape, dtype=F32):
        if name not in inputs:
            inputs[name] = nc.dram_tensor(name, list(shape), dtype, kind="ExternalInput")
        return inputs[name]

    x_in = inp("x_enc", (1024, OUT_V))
    out_pred = nc.dram_tensor("out_pred", [1, OUT_V], F32, kind="ExternalOutput")
    out_gt = nc.dram_tensor("out_gt", [1, OUT_V], F32, kind="ExternalOutput")
    dbg_names = []

    _scr = {}

    def layer_scratch(tag):
        if tag not in _scr:
            gb = [nc.dram_tensor(f"gbuf_{tag}{i}", [128 * (GW + 1) + 128], F32)
                  for i in range(8)]
            w2d = nc.dram_tensor(f"w2d_{tag}", [128 + 2 * LEN + 256], F32)
            w2r = nc.dram_tensor(f"w2rep_{tag}",
                                 [127 * (PITCH + 1) + 128 + 2 * LEN + 256 + 64], F32)
            _scr[tag] = (gb, w2d, w2r)
        return _scr[tag]

    rowbuf_dram = nc.dram_tensor("rowbuf_dram", [4 * D], F32)

    with TileContext(nc) as tc:
      with tc.tile_pool(name="psA", bufs=6, space="PSUM") as psA, \
           tc.tile_pool(name="psR", bufs=1, space="PSUM") as psR, \
           tc.tile_pool(name="persist", bufs=1) as pp, \
           tc.tile_pool(name="consts", bufs=1) as cp:

        def pa(shape):
            return psA.tile(list(shape), F32, tag="psA", name="psA_t")

        def pr(shape):
            return psR.tile(list(shape), F32, tag="psR", name="psR_t")

        ident = cp.tile([128, 128], F32)
        nc.sync.dma_start(out=ident[:], in_=inp("ident", (128, 128))[:])
        ones_col = cp.tile([128, 1], F32)
        nc.sync.dma_start(out=ones_col[:], in_=inp("ones_col", (128, 1))[:])
        inv512_col = cp.tile([128, 1], F32)
        nc.sync.dma_start(out=inv512_col[:], in_=inp("inv512_col", (128, 1))[:])
        ones_row = cp.tile([1, 128], F32)
        nc.sync.dma_start(out=ones_row[:], in_=inp("ones_row", (1, 128))[:])
        eps_col = cp.tile([128, 1], F32)
        nc.vector.memset(eps_col[:], 1e-5 * 512.0)

        def dbg_dump(name, tile_ap, shape):
            if name in dbg:
                h = nc.dram_tensor("dbg_" + name, list(shape), F32, kind="ExternalOutput")
                dbg_names.append("dbg_" + name)
                nc.sync.dma_start(out=h[:], in_=tile_ap)

        # ------------- helpers -------------
        def transpose_tiles(pool, x_sb, L, TP, tag):
            """x_sb normal tile-major [128, nt*512] -> xT [128, 4, TP] (c-major)."""
            xT = pool.tile([128, 4, TP], F32, tag=tag)
            for ti, (t0, tl) in enumerate(tiles_of(L)):
                for cc in range(4):
                    pt = pa([128, 128])
                    nc.tensor.transpose(pt[:, 0:tl],
                                        x_sb[0:tl, 512 * ti + 128 * cc:512 * ti + 128 * (cc + 1)],
                                        ident[0:tl, 0:tl])
                    if cc % 2 == 0:
                        nc.vector.tensor_copy(xT[:, cc, t0:t0 + tl], pt[:, 0:tl])
                    else:
                        nc.scalar.activation(xT[:, cc, t0:t0 + tl], pt[:, 0:tl], AF.Copy)
            return xT

        def projT(pool, wT_sb, bias_sb, xT, L, TP, tag):
            """out^T[f, t] (f-major [128, 4, TP])."""
            oT = pool.tile([128, 4, TP], F32, tag=tag)
            nts = [(n0, min(512, L - n0)) for n0 in range(0, L, 512)]
            for fc in range(4):
                for (n0, nl) in nts:
                    ps = pa([128, 512])
                    for cc in range(4):
                        nc.tensor.matmul(ps[:, 0:nl],
                                         wT_sb[:, cc, 128 * fc:128 * (fc + 1)],
                                         xT[:, cc, n0:n0 + nl],
                                         start=(cc == 0), stop=(cc == 3))
                    nc.scalar.activation(oT[:, fc, n0:n0 + nl], ps[:, 0:nl],
                                         AF.Identity, bias=bias_sb[:, fc:fc + 1], scale=1.0)
            return oT

        def proj_normal(pool, xT, w_sb, brow_sb, L, tag):
            """out[t, f] normal tile-major; bias row added via K=1 matmul."""
            nt = len(tiles_of(L))
            o = pool.tile([128, nt * 512], F32, tag=tag)
            for ti, (t0, tl) in enumerate(tiles_of(L)):
                ps = pa([128, 512])
                for cc in range(4):
                    nc.tensor.matmul(ps[0:tl, :], xT[:, cc, t0:t0 + tl], w_sb[:, cc, :],
                                     start=(cc == 0), stop=False)
                nc.tensor.matmul(ps[0:tl, :], ones_row[:, 0:tl], brow_sb[:],
                                 start=False, stop=True)
                nc.vector.tensor_copy(o[0:tl, 512 * ti:512 * (ti + 1)], ps[0:tl, :])
            return o

        def gram_corr(statT, movT, L, tag):
            """PSUM row [1, LP]: corrswap[d] = sum_t <stat_t, mov_(t+d) mod L> / 512."""
            nt = len(tiles_of(L))
            LP = 1024 if L > 640 else 640
            crow_ps = pr([1, LP])
            with tc.tile_pool(name="gramsc", bufs=2) as gp:
                for ti, (t0, tl) in enumerate(tiles_of(L)):
                    ncol = tl + L - 1
                    gsb = gp.tile([128, GW], F32, tag="gsb")
                    for (n0, nl) in [(a, min(512, ncol - a)) for a in range(0, ncol, 512)]:
                        gps = pa([128, 512])
                        splits = [n0]
                        for wpt in (L - t0, 2 * L - t0):
                            if n0 < wpt < n0 + nl:
                                splits.append(wpt)
                        splits.append(n0 + nl)
                        for si in range(len(splits) - 1):
                            a, b = splits[si], splits[si + 1]
                            col = (t0 + a) % L
                            for fc in range(4):
                                nc.tensor.matmul(gps[0:tl, a - n0:b - n0],
                                                 statT[:, fc, t0:t0 + tl],
                                                 movT[:, fc, col:col + (b - a)],
                                                 start=(fc == 0), stop=(fc == 3))
                        nc.vector.tensor_copy(gsb[0:tl, n0:n0 + nl], gps[0:tl, 0:nl])
                    gd = layer_scratch(tag)[0][ti]
                    i_w = nc.sync.dma_start(out=rap(gd, 0, [[GW, tl], [1, ncol]]),
                                            in_=gsb[0:tl, 0:ncol])
                    rtb = gp.tile([128, LP], F32, tag="rtb")
                    i_r = nc.sync.dma_start(out=rtb[0:tl, 0:L],
                                            in_=rap(gd, 0, [[GW + 1, tl], [1, L]]))
                    add_dep_helper(i_r.ins, i_w.ins, True, "gbuf raw")
                    for (n0, nl) in [(a, min(512, L - a)) for a in range(0, L, 512)]:
                        nc.tensor.matmul(crow_ps[:, n0:n0 + nl],
                                         inv512_col[0:tl, :], rtb[0:tl, n0:n0 + nl],
                                         start=(ti == 0), stop=(ti == nt - 1))
            return crow_ps

        def filter_from_corr(dest_pool, crow_ps, L, topk, tag):
            """Builds circulant filter tiles from the corrswap PSUM row."""
            nt = len(tiles_of(L))
            LP = 1024 if L > 640 else 640
            circ = dest_pool.tile([128, (2 * nt - 1) * 128], F32, tag="circ")
            with tc.tile_pool(name=f"rowp_{tag}", bufs=1) as gp:
                crow = gp.tile([1, LP], F32, tag="crow")
                nc.vector.memset(crow[:], NEG)
                nc.vector.tensor_copy(crow[:, 0:L], crow_ps[:, 0:L])
                dbg_dump(f"{tag}_crow", crow[:], (1, LP))
                s1 = gp.tile([1, LP], F32, tag="s1")
                m1 = gp.tile([1, 8], F32, tag="m1")
                m2 = gp.tile([1, 8], F32, tag="m2")
                m3 = gp.tile([1, 8], F32, tag="m3")
                nc.vector.tensor_copy(s1[:], crow[:])
                nc.vector.max(m1[:], s1[:])
                nc.vector.match_replace(s1[:], m1[:], s1[:], NEG)
                nc.vector.max(m2[:], s1[:])
                nc.vector.match_replace(s1[:], m2[:], s1[:], NEG)
                nc.vector.max(m3[:], s1[:])
                tau = m3[:, (topk - 17):(topk - 16)]
                negm = gp.tile([1, 1], F32, tag="negm")
                nc.vector.tensor_scalar(negm[:], m1[:, 0:1], scalar1=-1.0,
                                        scalar2=None, op0=ALU.mult)
                ge = gp.tile([1, LP], F32, tag="ge")
                nc.vector.tensor_scalar(ge[:], crow[:], scalar1=tau,
                                        scalar2=None, op0=ALU.is_ge)
                ex = gp.tile([1, LP], F32, tag="ex")
                nc.scalar.activation(ex[:], crow[:], AF.Exp, bias=negm[:, 0:1], scale=1.0)
                nc.vector.tensor_tensor(ex[:], ex[:], ge[:], op=ALU.mult)
                z = gp.tile([1, 1], F32, tag="z")
                nc.vector.tensor_reduce(z[:], ex[:, 0:L], op=ALU.add, axis=AXX)
                rz = gp.tile([1, 1], F32, tag="rz")
                nc.vector.reciprocal(rz[:], z[:])
                W2 = 128 + 2 * L + 128
                w2row = gp.tile([1, W2], F32, tag="w2row")
                nc.vector.memset(w2row[:], 0.0)
                nc.vector.tensor_scalar(w2row[:, 128:128 + L], ex[:, 0:L],
                                        scalar1=rz[:, 0:1], scalar2=None, op0=ALU.mult)
                nc.vector.tensor_copy(w2row[:, 128 + L:128 + 2 * L], w2row[:, 128:128 + L])
                dbg_dump(f"{tag}_w2row", w2row[:], (1, W2))
                _, w2d_dram, w2rep = layer_scratch(tag)
                i1 = nc.sync.dma_start(out=rap(w2d_dram, 0, [[1, 1], [1, W2]]), in_=w2row[:])
                i2 = nc.sync.dma_start(out=rap(w2rep, 0, [[PITCH + 1, 128], [1, W2]]),
                                       in_=rap(w2d_dram, 0, [[0, 128], [1, W2]]))
                add_dep_helper(i2.ins, i1.ins, True, "w2d raw")
                for mi in range(2 * nt - 1):
                    m = mi - (nt - 1)
                    i3 = nc.sync.dma_start(
                        out=circ[:, 128 * mi:128 * (mi + 1)],
                        in_=rap(w2rep, 128 + L + 128 * m, [[PITCH, 128], [1, 128]]))
                    add_dep_helper(i3.ins, i2.ins, True, "w2rep raw")
                dbg_dump(f"{tag}_circ", circ[:], (128, (2 * nt - 1) * 128))
            return circ

        def roll_add(circ, vo_sb, x_sb, L, out_sb):
            tls = tiles_of(L)
            nt = len(tls)
            for ti, (t0, tl) in enumerate(tls):
                ps = pa([128, 512])
                for i, (s0, sl) in enumerate(tls):
                    mi = (ti - i) + (nt - 1)
                    nc.tensor.matmul(ps[0:tl, :],
                                     circ[0:sl, 128 * mi:128 * mi + tl],
                                     vo_sb[0:sl, 512 * i:512 * (i + 1)],
                                     start=(i == 0), stop=(i == nt - 1))
                nc.vector.tensor_tensor(out_sb[0:tl, 512 * ti:512 * (ti + 1)],
                                        x_sb[0:tl, 512 * ti:512 * (ti + 1)],
                                        ps[0:tl, :], op=ALU.add)

        def load_iat(pool, name, L, tag):
            nt = len(tiles_of(L))
            h = inp(name, (nt * 128, nt * 128))
            tiles, idx = {}, 0
            buf = pool.tile([128, (3 * nt - 2) * 128], F32, tag=tag)
            for k in range(nt):
                for i in (k - 1, k, k + 1):
                    if 0 <= i < nt and (i, k) not in tiles:
                        sl_ap = buf[:, 128 * idx:128 * (idx + 1)]
                        nc.sync.dma_start(out=sl_ap,
                                          in_=h[128 * i:128 * (i + 1), 128 * k:128 * (k + 1)])
                        tiles[(i, k)] = sl_ap
                        idx += 1
            return tiles

        def banded_seasonal(iat_tiles, x_sb, L, out_sb):
            tls = tiles_of(L)
            for k, (t0, tl) in enumerate(tls):
                ps = pa([128, 512])
                ks = [i for i in (k - 1, k, k + 1) if 0 <= i < len(tls)]
                for j, i in enumerate(ks):
                    s0, sl = tls[i]
                    nc.tensor.matmul(ps[0:tl, :],
                                     iat_tiles[(i, k)][0:sl, 0:tl],
                                     x_sb[0:sl, 512 * i:512 * (i + 1)],
                                     start=(j == 0), stop=(j == len(ks) - 1))
                nc.vector.tensor_copy(out_sb[0:tl, 512 * k:512 * (k + 1)], ps[0:tl, :])

        def ffn_block(pool, x_sb, L, TP, c1name, c2name, tag):
            """returns out_sb = x + gelu(x@c1.T)@c2.T (from pool)."""
            nt = len(tiles_of(L))
            out_sb = pool.tile([128, nt * 512], F32, tag="xbig", name=f"ffnout_{tag}")
            with tc.tile_pool(name=f"ffn_{tag}", bufs=1) as fp:
                xT = transpose_tiles(fp, x_sb, L, TP, f"ffnxT_{tag}")
                c1 = fp.tile([128, 4, DFF], F32, tag="c1w")
                nc.sync.dma_start(out=c1[:],
                                  in_=inp(c1name, (D, DFF))[:].rearrange("(a p) f -> p a f", p=128))
                c2 = fp.tile([128, 16, 512], BF16, tag="c2w")
                nc.sync.dma_start(out=c2[:],
                                  in_=inp(c2name, (DFF, D), BF16)[:].rearrange("(a p) f -> p a f", p=128))
                groups = [list(range(g, min(g + 4, nt))) for g in range(0, nt, 4)]
                for grp in groups:
                    g0, gl = grp[0] * 128, sum(tiles_of(L)[i][1] for i in grp)
                    hT = fp.tile([128, 16, 512], BF16, tag="hT")
                    for fc in range(16):
                        for (n0, nl) in [(a, min(512, gl - a)) for a in range(0, gl, 512)]:
                            ps = pa([128, 512])
                            for cc in range(4):
                                nc.tensor.matmul(ps[:, 0:nl],
                                                 c1[:, cc, 128 * fc:128 * (fc + 1)],
                                                 xT[:, cc, g0 + n0:g0 + n0 + nl],
                                                 start=(cc == 0), stop=(cc == 3))
                            nc.scalar.activation(hT[:, fc, n0:n0 + nl], ps[:, 0:nl],
                                                 AF.Gelu_apprx_tanh)
                    for ti in grp:
                        t0, tl = tiles_of(L)[ti]
                        ps = pa([128, 512])
                        for fc in range(16):
                            nc.tensor.matmul(ps[0:tl, :],
                                             hT[:, fc, t0 - g0:t0 - g0 + tl],
                                             c2[:, fc, :],
                                             start=(fc == 0), stop=(fc == 15))
                        nc.vector.tensor_tensor(out_sb[0:tl, 512 * ti:512 * (ti + 1)],
                                                x_sb[0:tl, 512 * ti:512 * (ti + 1)],
                                                ps[0:tl, :], op=ALU.add)
            return out_sb

        def my_layernorm(pool, x_sb, L, tag):
            nt = len(tiles_of(L))
            out_sb = pool.tile([128, nt * 512], F32, tag=f"mln_{tag}")
            mrow_ps = pr([1, 512])
            with tc.tile_pool(name=f"ln_{tag}", bufs=2) as lp:
                for ti, (t0, tl) in enumerate(tiles_of(L)):
                    xt = x_sb[0:tl, 512 * ti:512 * (ti + 1)]
                    mean = lp.tile([128, 1], F32, tag="lnmean")
                    nc.vector.tensor_reduce(mean[0:tl, :], xt, op=ALU.add, axis=AXX)
                    nc.vector.tensor_scalar(mean[0:tl, :], mean[0:tl, :],
                                            scalar1=1.0 / 512.0, scalar2=None, op0=ALU.mult)
                    xc = lp.tile([128, 512], F32, tag="lnxc")
                    nc.vector.tensor_scalar(xc[0:tl, :], xt, scalar1=mean[0:tl, :],
                                            scalar2=None, op0=ALU.subtract)
                    ssq = lp.tile([128, 1], F32, tag="lnssq")
                    sqt = lp.tile([128, 512], F32, tag="lnsq")
                    nc.scalar.activation(sqt[0:tl, :], xc[0:tl, :], AF.Square,
                                         accum_out=ssq[0:tl, :])
                    rstd = lp.tile([128, 1], F32, tag="lnrstd")
                    nc.scalar.activation(rstd[0:tl, :], ssq[0:tl, :], AF.Sqrt,
                                         bias=eps_col[0:tl, :], scale=1.0)
                    nc.vector.reciprocal(rstd[0:tl, :], rstd[0:tl, :])
                    xh = out_sb[0:tl, 512 * ti:512 * (ti + 1)]
                    nc.vector.tensor_scalar(xh, xc[0:tl, :], scalar1=rstd[0:tl, :],
                                            scalar2=math.sqrt(512.0), op0=ALU.mult,
                                            op1=ALU.mult)
                    nc.tensor.matmul(mrow_ps[:], ones_col[0:tl, :], xh,
                                     start=(ti == 0), stop=(ti == nt - 1))
                negm_row = lp.tile([1, 512], F32, tag="lnnegm")
                nc.vector.tensor_scalar(negm_row[:], mrow_ps[:], scalar1=-1.0 / L,
                                        scalar2=None, op0=ALU.mult)
                for ti, (t0, tl) in enumerate(tiles_of(L)):
                    bps = pa([128, 512])
                    nc.tensor.matmul(bps[0:tl, :], ones_row[:, 0:tl], negm_row[:],
                                     start=True, stop=True)
                    nc.vector.tensor_tensor(out_sb[0:tl, 512 * ti:512 * (ti + 1)],
                                            out_sb[0:tl, 512 * ti:512 * (ti + 1)],
                                            bps[0:tl, :], op=ALU.add)
            return out_sb

        def ac_layer(pool, x_q_sb, xT_q, kvT_src, L, topk, wtag, out_sb):
            TP = 1024 if L > 640 else 640
            with tc.tile_pool(name=f"acw_{wtag}", bufs=1) as awp:
                wq_sb = awp.tile([128, 4, 512], F32, tag="wq")
                nc.sync.dma_start(out=wq_sb[:],
                                  in_=inp(f"{wtag}_wqT", (D, D))[:].rearrange("(a p) f -> p a f", p=128))
                wk_sb = awp.tile([128, 4, 512], F32, tag="wk")
                nc.sync.dma_start(out=wk_sb[:],
                                  in_=inp(f"{wtag}_wkT", (D, D))[:].rearrange("(a p) f -> p a f", p=128))
                wvo_sb = awp.tile([128, 4, 512], F32, tag="wvo")
                nc.sync.dma_start(out=wvo_sb[:],
                                  in_=inp(f"{wtag}_wvoT", (D, D))[:].rearrange("(a p) f -> p a f", p=128))
                bq_sb = awp.tile([128, 4], F32, tag="bq")
                nc.sync.dma_start(out=bq_sb[:], in_=inp(f"{wtag}_bq", (128, 4))[:])
                bk_sb = awp.tile([128, 4], F32, tag="bk")
                nc.sync.dma_start(out=bk_sb[:], in_=inp(f"{wtag}_bk", (128, 4))[:])
                bvo_sb = awp.tile([1, D], F32, tag="bvo")
                nc.sync.dma_start(out=bvo_sb[:], in_=inp(f"{wtag}_bvo", (1, D))[:])

                with tc.tile_pool(name=f"accirc_{wtag}", bufs=1) as circp:
                    with tc.tile_pool(name=f"acqk_{wtag}", bufs=1) as qkp:
                        kT = projT(qkp, wk_sb, bk_sb, kvT_src, L, TP, "kT")
                        qT = projT(qkp, wq_sb, bq_sb, xT_q, L, TP, "qT")
                        crow_ps = gram_corr(qT, kT, L, wtag)
                    circ = filter_from_corr(circp, crow_ps, L, topk, wtag)
                    with tc.tile_pool(name=f"acvo_{wtag}", bufs=1) as vop:
                        vo_sb = proj_normal(vop, kvT_src, wvo_sb, bvo_sb, L, "vo")
                        roll_add(circ, vo_sb, x_q_sb, L, out_sb)

        # =========================================================
        # Phase 0: LIN + inits + embeddings
        # =========================================================
        xn = pp.tile([128, 8, OUT_V], F32, tag="xn")
        mean_row = pp.tile([1, OUT_V], F32, tag="meanrow")
        xd_sb = pp.tile([128, 5 * 512], F32, tag="x_dec_stream")

        with tc.tile_pool(name="ph0", bufs=1) as p0:
            xr = p0.tile([128, 8, OUT_V], F32, tag="xraw")
            nc.sync.dma_start(out=xr[:], in_=x_in[:].rearrange("(a p) c -> p a c", p=128))
            sum_ps = pr([1, OUT_V])
            for a in range(8):
                nc.tensor.matmul(sum_ps[:], ones_col[:], xr[:, a, :],
                                 start=(a == 0), stop=(a == 7))
            mu = p0.tile([1, OUT_V], F32, tag="mu")
            nc.vector.tensor_scalar(mu[:], sum_ps[:], scalar1=1.0 / 1024.0,
                                    scalar2=None, op0=ALU.mult)
            sq = p0.tile([128, 8, OUT_V], F32, tag="xsq")
            nc.scalar.activation(sq[:], xr[:], AF.Square)
            ssq_ps = pr([1, OUT_V])
            for a in range(8):
                nc.tensor.matmul(ssq_ps[:], ones_col[:], sq[:, a, :],
                                 start=(a == 0), stop=(a == 7))
            var = p0.tile([1, OUT_V], F32, tag="var")
            nc.vector.tensor_scalar(var[:], ssq_ps[:], scalar1=1.0 / 1024.0,
                                    scalar2=None, op0=ALU.mult)
            musq = p0.tile([1, OUT_V], F32, tag="musq")
            nc.scalar.activation(musq[:], mu[:], AF.Square)
            nc.vector.tensor_tensor(var[:], var[:], musq[:], op=ALU.subtract)
            std = p0.tile([1, OUT_V], F32, tag="std")
            nc.scalar.activation(std[:], var[:], AF.Sqrt)
            rstd = p0.tile([1, OUT_V], F32, tag="rstd")
            nc.vector.reciprocal(rstd[:], std[:])
            negmu = p0.tile([1, OUT_V], F32, tag="negmu")
            nc.vector.tensor_scalar(negmu[:], mu[:], scalar1=-1.0, scalar2=None, op0=ALU.mult)
            bmu_ps = pa([128, OUT_V])
            nc.tensor.matmul(bmu_ps[:], ones_row[:], negmu[:], start=True, stop=True)
            bmu = p0.tile([128, OUT_V], F32, tag="bmusb")
            nc.vector.tensor_copy(bmu[:], bmu_ps[:])
            brs_ps = pa([128, OUT_V])
            nc.tensor.matmul(brs_ps[:], ones_row[:], rstd[:], start=True, stop=True)
            brs = p0.tile([128, OUT_V], F32, tag="brssb")
            nc.vector.tensor_copy(brs[:], brs_ps[:])
            for a in range(8):
                nc.vector.tensor_tensor(xn[:, a, :], xr[:, a, :], bmu[:], op=ALU.add)
                nc.vector.tensor_tensor(xn[:, a, :], xn[:, a, :], brs[:], op=ALU.mult)
            gt_row = p0.tile([1, OUT_V], F32, tag="gtrow")
            nc.sync.dma_start(out=gt_row[:], in_=xn[127:128, 7, :])
            nc.sync.dma_start(out=out_gt[:], in_=gt_row[:])

            xm_ps = pr([1, OUT_V])
            for a in range(8):
                pl = 128 if a < 7 else 127
                nc.tensor.matmul(xm_ps[:], ones_col[0:pl, :], xn[0:pl, a, :],
                                 start=(a == 0), stop=(a == 7))
            nc.vector.tensor_scalar(mean_row[:], xm_ps[:], scalar1=1.0 / 1023.0,
                                    scalar2=None, op0=ALU.mult)

            xinT = p0.tile([21, 1026], F32, tag="xinT")
            for a in range(8):
                pl = 128 if a < 7 else 127
                tp_ = pa([21, 128])
                nc.tensor.transpose(tp_[:, 0:pl], xn[0:pl, a, :], ident[0:pl, 0:pl])
                nc.vector.tensor_copy(xinT[:, 1 + 128 * a:1 + 128 * a + pl], tp_[:, 0:pl])
            nc.vector.tensor_copy(xinT[:, 0:1], xinT[:, 1023:1024])
            nc.vector.tensor_copy(xinT[:, 1024:1025], xinT[:, 1:2])

            sb_bt = inp("sinit_bt", (8 * 128, 512))
            sinit = p0.tile([128, 5, OUT_V], F32, tag="sinit")
            with tc.tile_pool(name="sinitl", bufs=2) as slp:
                for k in range(4):
                    ps = pa([128, OUT_V])
                    lo = (511 + 128 * k - 12) // 128
                    hi = min(7, (511 + 128 * k + 127 + 12) // 128)
                    rng = list(range(lo, hi + 1))
                    for j, i in enumerate(rng):
                        sl = 128 if i < 7 else 127
                        bt_t = slp.tile([128, 128], F32, tag="btt")
                        nc.sync.dma_start(out=bt_t[:],
                                          in_=sb_bt[128 * i:128 * (i + 1), 128 * k:128 * (k + 1)])
                        nc.tensor.matmul(ps[:], bt_t[0:sl, :], xn[0:sl, i, :],
                                         start=(j == 0), stop=(j == len(rng) - 1))
                    nc.vector.tensor_copy(sinit[:, k, :], ps[:])
            nc.vector.memset(sinit[:, 4, :], 0.0)

            sinitT = p0.tile([21, 516], F32, tag="sinitT")
            nc.vector.memset(sinitT[:], 0.0)
            for k in range(4):
                tp_ = pa([21, 128])
                nc.tensor.transpose(tp_[:], sinit[:, k, :], ident[:])
                nc.vector.tensor_copy(sinitT[:, 1 + 128 * k:1 + 128 * (k + 1)], tp_[:])
            nc.vector.tensor_copy(sinitT[:, 514:515], sinitT[:, 1:2])

            x_sb = pp.tile([128, 8 * 512], F32, tag="x_enc_stream")
            embw = []
            for i in range(3):
                t = p0.tile([OUT_V, D], F32, tag=f"embw{i}")
                nc.sync.dma_start(out=t[:], in_=inp(f"enc_emb_{i}", (OUT_V, D))[:])
                embw.append(t)
            for ti, (t0, tl) in enumerate(tiles_of(LEN)):
                ps = pa([128, 512])
                for d_i in range(3):
                    nc.tensor.matmul(ps[0:tl, :], xinT[:, t0 + d_i:t0 + d_i + tl],
                                     embw[d_i][:], start=(d_i == 0), stop=(d_i == 2))
                nc.vector.tensor_copy(x_sb[0:tl, 512 * ti:512 * (ti + 1)], ps[0:tl, :])

            dembw = []
            for i in range(3):
                t = p0.tile([OUT_V, D], F32, tag=f"dembw{i}")
                nc.sync.dma_start(out=t[:], in_=inp(f"dec_emb_{i}", (OUT_V, D))[:])
                dembw.append(t)
            pe_in = inp("pos_emb", (LDEC, D))
            with tc.tile_pool(name="peld", bufs=2) as pel:
                for ti, (t0, tl) in enumerate(tiles_of(LDEC)):
                    ps = pa([128, 512])
                    for d_i in range(3):
                        nc.tensor.matmul(ps[0:tl, :], sinitT[:, t0 + d_i:t0 + d_i + tl],
                                         dembw[d_i][:], start=(d_i == 0), stop=(d_i == 2))
                    pet = pel.tile([128, 512], F32, tag="pet")
                    nc.sync.dma_start(out=pet[0:tl, :], in_=pe_in[t0:t0 + tl, :])
                    nc.vector.tensor_tensor(xd_sb[0:tl, 512 * ti:512 * (ti + 1)],
                                            ps[0:tl, :], pet[0:tl, :], op=ALU.add)

        dbg_dump("emb", x_sb[:], (128, 8 * 512))
        dbg_dump("demb", xd_sb[:], (128, 5 * 512))

        # =========================================================
        # Encoder
        # =========================================================
        with tc.tile_pool(name="enc_scope", bufs=1) as esc:
            iat_e = load_iat(esc, "iat_e", LEN, "iat_e")
            for li in range(2):
                with tc.tile_pool(name=f"encl{li}", bufs=1) as lp:
                    x2 = lp.tile([128, 8 * 512], F32, tag="xbig")
                    with tc.tile_pool(name=f"encT{li}", bufs=1) as tpp_:
                        xT = transpose_tiles(tpp_, x_sb, LEN, 1024, "xTe")
                        ac_layer(lp, x_sb, xT, xT, LEN, 20, f"enc{li}", x2)
                    dbg_dump(f"acout{li}", x2[:], (128, 8 * 512))
                    x2s = lp.tile([128, 8 * 512], F32, tag="x2s")
                    banded_seasonal(iat_e, x2, LEN, x2s)
                    x3 = ffn_block(lp, x2s, LEN, 1024, f"enc{li}_conv1T", f"enc{li}_conv2R", "e")
                    banded_seasonal(iat_e, x3, LEN, x_sb)
                dbg_dump(f"encl{li}", x_sb[:], (128, 8 * 512))
        enc_out = my_layernorm(pp, x_sb, LEN, "encln")
        dbg_dump("encout", enc_out[:], (128, 8 * 512))

        # =========================================================
        # Decoder
        # =========================================================
        with tc.tile_pool(name="dec_scope", bufs=1) as dsc:
            iat_d = load_iat(dsc, "iat_d", LDEC, "iat_d")
            encT = transpose_tiles(dsc, enc_out, LDEC, 640, "encT")
            tsum = dsc.tile([3, 512], F32, tag="tsum")
            arows_h = inp("a_rows_d", (5 * 128, 3))

            def trend_rows(x_cur, first):
                trow_ps = pr([3, 512])
                with tc.tile_pool(name="trl", bufs=2) as trl:
                    tls = tiles_of(LDEC)
                    for j, (s0, sl) in enumerate(tls):
                        at = trl.tile([128, 3], F32, tag="att")
                        nc.sync.dma_start(out=at[:], in_=arows_h[128 * j:128 * (j + 1), :])
                        nc.tensor.matmul(trow_ps[:], at[0:sl, :],
                                         x_cur[0:sl, 512 * j:512 * (j + 1)],
                                         start=(j == 0), stop=(j == len(tls) - 1))
                if first:
                    nc.vector.tensor_copy(tsum[:], trow_ps[:])
                else:
                    nc.vector.tensor_tensor(tsum[:], tsum[:], trow_ps[:], op=ALU.add)

            xd2 = dsc.tile([128, 5 * 512], F32, tag="xdbig")
            with tc.tile_pool(name="decT1", bufs=1) as dt1:
                xdT = transpose_tiles(dt1, xd_sb, LDEC, 640, "xdT")
                ac_layer(dsc, xd_sb, xdT, xdT, LDEC, 18, "dec_sa", xd2)
            xd2s = dsc.tile([128, 5 * 512], F32, tag="xd2s")
            banded_seasonal(iat_d, xd2, LDEC, xd2s)
            trend_rows(xd2, True)
            dbg_dump("xd2s", xd2s[:], (128, 5 * 512))

            xd3 = dsc.tile([128, 5 * 512], F32, tag="xdbig", name="xd3")
            with tc.tile_pool(name="decT2", bufs=1) as dt2:
                xd2sT = transpose_tiles(dt2, xd2s, LDEC, 640, "xd2sT")
                ac_layer(dsc, xd2s, xd2sT, encT, LDEC, 18, "dec_ca", xd3)
            xd3s = dsc.tile([128, 5 * 512], F32, tag="xd3s")
            banded_seasonal(iat_d, xd3, LDEC, xd3s)
            trend_rows(xd3, False)
            dbg_dump("xd3s", xd3s[:], (128, 5 * 512))

            xd4 = ffn_block(dsc, xd3s, LDEC, 640, "dec_conv1T", "dec_conv2R", "d")
            xd4s = dsc.tile([128, 5 * 512], F32, tag="xd4s")
            banded_seasonal(iat_d, xd4, LDEC, xd4s)
            trend_rows(xd4, False)

            dec_ln = my_layernorm(dsc, xd4s, LDEC, "decln")
            dbg_dump("decln", dec_ln[:], (128, 5 * 512))

            fin_row = dsc.tile([1, 512], F32, tag="finrow")
            nc.vector.tensor_copy(fin_row[:], dec_ln[0:1, 4 * 512:5 * 512])
            nc.sync.dma_start(out=rap(rowbuf_dram, 0, [[1, 1], [1, 512]]), in_=fin_row[:])
            nc.sync.dma_start(out=rap(rowbuf_dram, 512, [[512, 3], [1, 512]]), in_=tsum[:])
            fcol = dsc.tile([128, 4], F32, tag="fcol")
            nc.sync.dma_start(out=fcol[:], in_=rap(rowbuf_dram, 0, [[1, 128], [128, 4]]))
            tcol = dsc.tile([128, 12], F32, tag="tcol")
            nc.sync.dma_start(out=tcol[:], in_=rap(rowbuf_dram, 512, [[1, 128], [128, 12]]))

            with tc.tile_pool(name="head", bufs=1) as hp:
                pw = hp.tile([128, 4, OUT_V], F32, tag="pw")
                nc.sync.dma_start(out=pw[:],
                                  in_=inp("proj_wT", (D, OUT_V))[:].rearrange("(a p) f -> p a f", p=128))
                head_ps = pr([1, OUT_V])
                for cc in range(4):
                    nc.tensor.matmul(head_ps[:], fcol[:, cc:cc + 1], pw[:, cc, :],
                                     start=(cc == 0), stop=False)
                tw = []
                for i in range(3):
                    t = hp.tile([128, 4, OUT_V], F32, tag=f"tw{i}")
                    nc.sync.dma_start(out=t[:],
                                      in_=inp(f"trend_w{i}", (D, OUT_V))[:].rearrange("(a p) f -> p a f", p=128))
                    tw.append(t)
                pairs = [(1, 0), (2, 1), (0, 2)]
                for pi, (j, i) in enumerate(pairs):
                    for cc in range(4):
                        nc.tensor.matmul(head_ps[:],
                                         tcol[:, 4 * j + cc:4 * j + cc + 1],
                                         tw[i][:, cc, :],
                                         start=False, stop=(pi == 2 and cc == 3))
                pb = hp.tile([1, OUT_V], F32, tag="pb")
                nc.sync.dma_start(out=pb[:], in_=inp("proj_b", (1, OUT_V))[:])
                fin = hp.tile([1, OUT_V], F32, tag="fin")
                nc.vector.tensor_tensor(fin[:], head_ps[:], pb[:], op=ALU.add)
                nc.vector.tensor_tensor(fin[:], fin[:], mean_row[:], op=ALU.add)
                nc.sync.dma_start(out=out_pred[:], in_=fin[:])

    nc.compile()
    return nc, inputs, dbg_names


_CACHE = {}


def kernel(x_enc, drop, params, dbg=(), _want_results=False, trace=False):
    x_enc = np.asarray(x_enc, np.float32)
    key = tuple(sorted(dbg))
    if key not in _CACHE:
        _CACHE[key] = build_program(dbg=dbg)
    nc, inputs, dbg_names = _CACHE[key]
    consts = host_consts(params)
    in_maps = []
    for b in range(BATCH):
        m = {"x_enc": np.ascontiguousarray(x_enc[b])}
        for name in inputs:
            if name != "x_enc":
                m[name] = consts[name]
        in_maps.append(m)
    res = run_bass_kernel_spmd(nc, in_maps, core_ids=list(range(BATCH)), trace=trace)
    pred = np.stack([r["out_pred"] for r in res.results])
    gt = np.stack([r["out_gt"] for r in res.results])
    if _want_results:
        return (pred, gt), res
    return pred, gt


# revision 20
# speedup vs baseline: 1.0218x; 1.0218x over previous
"""Autoformer forward pass on Trainium2 (Bass/Tile), data-parallel over batch (8 cores).

- One NeuronCore per batch element (B=8), same NEFF on all cores (SPMD).
- Residual stream in NORMAL layout [t, c] (t on partitions, tile-major), PE transposes
  to [c, t] for channel-contraction matmuls.
- Autocorrelation mean-corr via Gram matrix with swapped roles:
  corrswap[d] = sum_t <q_t, k_(t+d) mod L> / 512  (= corr[(L-d) mod L]),
  diagonal (skew) reduction via DRAM round-trip with stride-(W+1) AP.
- top-k via threshold: tau = k-th largest (3 rounds of max8 + match_replace), softmax
  weights = exp(corr - max) * (corr >= tau) / Z. Filter applied as circulant matmul whose
  stationary tiles come from a DRAM "staircase"-replicated filter row.
- series_decomp as banded matmul with host-built (I-A)^T.
- Only last output timestep needed -> decoder trend path reduced to 3 rows.
"""

import math
import numpy as np
import ml_dtypes

import bass_rust
import concourse.bacc as bacc
import concourse.mybir as mybir
from concourse.tile import TileContext, add_dep_helper
from concourse.bass_utils import run_bass_kernel_spmd

F32 = mybir.dt.float32
BF16 = mybir.dt.bfloat16
AF = mybir.ActivationFunctionType
ALU = mybir.AluOpType
AXX = mybir.AxisListType.X

OUT_V = 21
D = 512
DFF = 2048
LEN = 1023
LDEC = 513
MA = 25
BATCH = 8
NEG = -3.0e38
GW = 1536              # gram scratch row width
PITCH = 8192           # staircase pitch


def tiles_of(L):
    out, t0 = [], 0
    while t0 < L:
        out.append((t0, min(128, L - t0)))
        t0 += 128
    return out


def rap(t, offset, dims):
    return bass_rust.AP(t, offset, [list(d) for d in dims])


def moving_avg_matrix(L, k=MA):
    A = np.zeros((L, L), np.float64)
    pad = (k - 1) // 2
    for t in range(L):
        for d in range(-pad, pad + 1):
            A[t, min(max(t + d, 0), L - 1)] += 1.0 / k
    return A.astype(np.float32)


def host_consts(params):
    c = {}
    c["ident"] = np.eye(128, dtype=np.float32)
    c["ones_col"] = np.ones((128, 1), np.float32)
    c["inv512_col"] = np.full((128, 1), 1.0 / 512.0, np.float32)
    c["ones_row"] = np.ones((1, 128), np.float32)

    for L, tag in ((LEN, "e"), (LDEC, "d")):
        A = moving_avg_matrix(L)
        IA = np.eye(L, dtype=np.float32) - A
        nt = (L + 127) // 128
        IAT = np.zeros((nt * 128, nt * 128), np.float32)
        IAT[:L, :L] = IA.T
        c[f"iat_{tag}"] = IAT
        if tag == "d":
            RT = np.zeros((nt * 128, 3), np.float32)
            RT[:L, :] = A[[0, L - 2, L - 1], :].T
            c["a_rows_d"] = RT

    A = moving_avg_matrix(LEN)
    IA = np.eye(LEN, dtype=np.float32) - A
    BT = np.zeros((8 * 128, 512), np.float32)
    BT[:LEN, :] = IA[511:1023, :].T
    c["sinit_bt"] = BT

    def convw(w):
        w = np.asarray(w)
        return [np.ascontiguousarray(w[:, :, i].T, np.float32) for i in range(3)]

    for i, wd in enumerate(convw(params["enc_emb"])):
        c[f"enc_emb_{i}"] = wd
    for i, wd in enumerate(convw(params["dec_emb"])):
        c[f"dec_emb_{i}"] = wd

    pos = np.arange(LDEC)[:, None].astype(np.float32)
    div = np.exp(np.arange(0, D, 2).astype(np.float32) * -(math.log(10000.0) / D))
    pe = np.zeros((LDEC, D), np.float32)
    pe[:, 0::2] = np.sin(pos * div)
    pe[:, 1::2] = np.cos(pos * div)
    c["pos_emb"] = pe

    def ac_consts(pref, p, pre):
        wq, bq = np.asarray(p[pre + "wq"]), np.asarray(p[pre + "bq"])
        wk, bk = np.asarray(p[pre + "wk"]), np.asarray(p[pre + "bk"])
        wv, bv = np.asarray(p[pre + "wv"]), np.asarray(p[pre + "bv"])
        wo, bo = np.asarray(p[pre + "wo"]), np.asarray(p[pre + "bo"])
        c[f"{pref}_wqT"] = np.ascontiguousarray(wq.T, np.float32)
        c[f"{pref}_wkT"] = np.ascontiguousarray(wk.T, np.float32)
        c[f"{pref}_wvoT"] = np.ascontiguousarray((wo @ wv).T, np.float32)
        c[f"{pref}_bq"] = np.ascontiguousarray(bq.reshape(4, 128).T, np.float32)
        c[f"{pref}_bk"] = np.ascontiguousarray(bk.reshape(4, 128).T, np.float32)
        c[f"{pref}_bvo"] = (wo @ bv + bo).reshape(1, D).astype(np.float32)

    for li, p in enumerate(params["enc_layers"]):
        ac_consts(f"enc{li}", p, "")
        c[f"enc{li}_conv1T"] = np.ascontiguousarray(np.asarray(p["conv1"]).T, np.float32)
        c[f"enc{li}_conv2R"] = np.ascontiguousarray(np.asarray(p["conv2"]).T).astype(ml_dtypes.bfloat16)
    dp = params["dec_layers"][0]
    ac_consts("dec_sa", dp, "sa_")
    ac_consts("dec_ca", dp, "ca_")
    c["dec_conv1T"] = np.ascontiguousarray(np.asarray(dp["conv1"]).T, np.float32)
    c["dec_conv2R"] = np.ascontiguousarray(np.asarray(dp["conv2"]).T).astype(ml_dtypes.bfloat16)
    tw = np.asarray(dp["trend_proj"])
    for i in range(3):
        c[f"trend_w{i}"] = np.ascontiguousarray(tw[:, :, i].T, np.float32)

    c["proj_wT"] = np.ascontiguousarray(np.asarray(params["proj_w"]).T, np.float32)
    c["proj_b"] = np.asarray(params["proj_b"]).reshape(1, OUT_V).astype(np.float32)
    return c


def build_program(dbg=()):
    nc = bacc.Bacc("TRN2", target_bir_lowering=False, debug=False)
    inputs = {}

    def inp(name, shape, dtype=F32):
        if name not in inputs:
            inputs[name] = nc.dram_tensor(name, list(shape), dtype, kind="ExternalInput")
        return inputs[name]

    x_in = inp("x_enc", (1024, OUT_V))
    out_pred = nc.dram_tensor("out_pred", [1, OUT_V], F32, kind="ExternalOutput")
    out_gt = nc.dram_tensor("out_gt", [1, OUT_V], F32, kind="ExternalOutput")
    dbg_names = []

    _scr = {}

    def layer_scratch(tag):
        if tag not in _scr:
            gb = [nc.dram_tensor(f"gbuf_{tag}{i}", [128 * (GW + 1) + 128], F32)
                  for i in range(8)]
            w2d = nc.dram_tensor(f"w2d_{tag}", [128 + 2 * LEN + 256], F32)
            w2r = nc.dram_tensor(f"w2rep_{tag}",
                                 [127 * (PITCH + 1) + 128 + 2 * LEN + 256 + 64], F32)
            _scr[tag] = (gb, w2d, w2r)
        return _scr[tag]

    rowbuf_dram = nc.dram_tensor("rowbuf_dram", [4 * D], F32)

    with TileContext(nc) as tc:
      with tc.tile_pool(name="psA", bufs=6, space="PSUM") as psA, \
           tc.tile_pool(name="psR", bufs=1, space="PSUM") as psR, \
           tc.tile_pool(name="persist", bufs=1) as pp, \
           tc.tile_pool(name="consts", bufs=1) as cp:

        def pa(shape):
            return psA.tile(list(shape), F32, tag="psA", name="psA_t")

        def pr(shape):
            return psR.tile(list(shape), F32, tag="psR", name="psR_t")

        ident = cp.tile([128, 128], F32)
        nc.sync.dma_start(out=ident[:], in_=inp("ident", (128, 128))[:])
        ones_col = cp.tile([128, 1], F32)
        nc.sync.dma_start(out=ones_col[:], in_=inp("ones_col", (128, 1))[:])
        inv512_col = cp.tile([128, 1], F32)
        nc.sync.dma_start(out=inv512_col[:], in_=inp("inv512_col", (128, 1))[:])
        ones_row = cp.tile([1, 128], F32)
        nc.sync.dma_start(out=ones_row[:], in_=inp("ones_row", (1, 128))[:])
        eps_col = cp.tile([128, 1], F32)
        nc.vector.memset(eps_col[:], 1e-5 * 512.0)

        def dbg_dump(name, tile_ap, shape):
            if name in dbg:
                h = nc.dram_tensor("dbg_" + name, list(shape), F32, kind="ExternalOutput")
                dbg_names.append("dbg_" + name)
                nc.sync.dma_start(out=h[:], in_=tile_ap)

        # ------------- helpers -------------
        def transpose_tiles(pool, x_sb, L, TP, tag):
            """x_sb normal tile-major [128, nt*512] -> xT [128, 4, TP] (c-major)."""
            xT = pool.tile([128, 4, TP], F32, tag=tag)
            for ti, (t0, tl) in enumerate(tiles_of(L)):
                for cc in range(4):
                    pt = pa([128, 128])
                    nc.tensor.transpose(pt[:, 0:tl],
                                        x_sb[0:tl, 512 * ti + 128 * cc:512 * ti + 128 * (cc + 1)],
                                        ident[0:tl, 0:tl])
                    if cc % 2 == 0:
                        nc.vector.tensor_copy(xT[:, cc, t0:t0 + tl], pt[:, 0:tl])
                    else:
                        nc.scalar.activation(xT[:, cc, t0:t0 + tl], pt[:, 0:tl], AF.Copy)
            return xT

        def projT(pool, wT_sb, bias_sb, xT, L, TP, tag):
            """out^T[f, t] (f-major [128, 4, TP])."""
            oT = pool.tile([128, 4, TP], F32, tag=tag)
            nts = [(n0, min(512, L - n0)) for n0 in range(0, L, 512)]
            for fc in range(4):
                for (n0, nl) in nts:
                    ps = pa([128, 512])
                    for cc in range(4):
                        nc.tensor.matmul(ps[:, 0:nl],
                                         wT_sb[:, cc, 128 * fc:128 * (fc + 1)],
                                         xT[:, cc, n0:n0 + nl],
                                         start=(cc == 0), stop=(cc == 3))
                    nc.scalar.activation(oT[:, fc, n0:n0 + nl], ps[:, 0:nl],
                                         AF.Identity, bias=bias_sb[:, fc:fc + 1], scale=1.0)
            return oT

        def proj_normal(pool, xT, w_sb, brow_sb, L, tag):
            """out[t, f] normal tile-major; bias row added via K=1 matmul."""
            nt = len(tiles_of(L))
            o = pool.tile([128, nt * 512], F32, tag=tag)
            for ti, (t0, tl) in enumerate(tiles_of(L)):
                ps = pa([128, 512])
                for cc in range(4):
                    nc.tensor.matmul(ps[0:tl, :], xT[:, cc, t0:t0 + tl], w_sb[:, cc, :],
                                     start=(cc == 0), stop=False)
                nc.tensor.matmul(ps[0:tl, :], ones_row[:, 0:tl], brow_sb[:],
                                 start=False, stop=True)
                nc.vector.tensor_copy(o[0:tl, 512 * ti:512 * (ti + 1)], ps[0:tl, :])
            return o

        def gram_corr(statT, movT, L, tag):
            """PSUM row [1, LP]: corrswap[d] = sum_t <stat_t, mov_(t+d) mod L> / 512."""
            nt = len(tiles_of(L))
            LP = 1024 if L > 640 else 640
            crow_ps = pr([1, LP])
            with tc.tile_pool(name="gramsc", bufs=2) as gp:
                for ti, (t0, tl) in enumerate(tiles_of(L)):
                    ncol = tl + L - 1
                    gsb = gp.tile([128, GW], F32, tag="gsb")
                    for (n0, nl) in [(a, min(512, ncol - a)) for a in range(0, ncol, 512)]:
                        gps = pa([128, 512])
                        splits = [n0]
                        for wpt in (L - t0, 2 * L - t0):
                            if n0 < wpt < n0 + nl:
                                splits.append(wpt)
                        splits.append(n0 + nl)
                        for si in range(len(splits) - 1):
                            a, b = splits[si], splits[si + 1]
                            col = (t0 + a) % L
                            for fc in range(4):
                                nc.tensor.matmul(gps[0:tl, a - n0:b - n0],
                                                 statT[:, fc, t0:t0 + tl],
                                                 movT[:, fc, col:col + (b - a)],
                                                 start=(fc == 0), stop=(fc == 3))
                        nc.vector.tensor_copy(gsb[0:tl, n0:n0 + nl], gps[0:tl, 0:nl])
                    gd = layer_scratch(tag)[0][ti]
                    i_w = nc.sync.dma_start(out=rap(gd, 0, [[GW, tl], [1, ncol]]),
                                            in_=gsb[0:tl, 0:ncol])
                    rtb = gp.tile([128, LP], F32, tag="rtb")
                    i_r = nc.sync.dma_start(out=rtb[0:tl, 0:L],
                                            in_=rap(gd, 0, [[GW + 1, tl], [1, L]]))
                    add_dep_helper(i_r.ins, i_w.ins, True, "gbuf raw")
                    for (n0, nl) in [(a, min(512, L - a)) for a in range(0, L, 512)]:
                        nc.tensor.matmul(crow_ps[:, n0:n0 + nl],
                                         inv512_col[0:tl, :], rtb[0:tl, n0:n0 + nl],
                                         start=(ti == 0), stop=(ti == nt - 1))
            return crow_ps

        def filter_from_corr(dest_pool, crow_ps, L, topk, tag):
            """Builds circulant filter tiles from the corrswap PSUM row."""
            nt = len(tiles_of(L))
            LP = 1024 if L > 640 else 640
            circ = dest_pool.tile([128, (2 * nt - 1) * 128], F32, tag="circ")
            with tc.tile_pool(name=f"rowp_{tag}", bufs=1) as gp:
                crow = gp.tile([1, LP], F32, tag="crow")
                nc.vector.memset(crow[:], NEG)
                nc.vector.tensor_copy(crow[:, 0:L], crow_ps[:, 0:L])
                dbg_dump(f"{tag}_crow", crow[:], (1, LP))
                s1 = gp.tile([1, LP], F32, tag="s1")
                m1 = gp.tile([1, 8], F32, tag="m1")
                m2 = gp.tile([1, 8], F32, tag="m2")
                m3 = gp.tile([1, 8], F32, tag="m3")
                nc.vector.tensor_copy(s1[:], crow[:])
                nc.vector.max(m1[:], s1[:])
                nc.vector.match_replace(s1[:], m1[:], s1[:], NEG)
                nc.vector.max(m2[:], s1[:])
                nc.vector.match_replace(s1[:], m2[:], s1[:], NEG)
                nc.vector.max(m3[:], s1[:])
                tau = m3[:, (topk - 17):(topk - 16)]
                negm = gp.tile([1, 1], F32, tag="negm")
                nc.vector.tensor_scalar(negm[:], m1[:, 0:1], scalar1=-1.0,
                                        scalar2=None, op0=ALU.mult)
                ge = gp.tile([1, LP], F32, tag="ge")
                nc.vector.tensor_scalar(ge[:], crow[:], scalar1=tau,
                                        scalar2=None, op0=ALU.is_ge)
                ex = gp.tile([1, LP], F32, tag="ex")
                nc.scalar.activation(ex[:], crow[:], AF.Exp, bias=negm[:, 0:1], scale=1.0)
                nc.vector.tensor_tensor(ex[:], ex[:], ge[:], op=ALU.mult)
                z = gp.tile([1, 1], F32, tag="z")
                nc.vector.tensor_reduce(z[:], ex[:, 0:L], op=ALU.add, axis=AXX)
                rz = gp.tile([1, 1], F32, tag="rz")
                nc.vector.reciprocal(rz[:], z[:])
                W2 = 128 + 2 * L + 128
                w2row = gp.tile([1, W2], F32, tag="w2row")
                nc.vector.memset(w2row[:], 0.0)
                nc.vector.tensor_scalar(w2row[:, 128:128 + L], ex[:, 0:L],
                                        scalar1=rz[:, 0:1], scalar2=None, op0=ALU.mult)
                nc.vector.tensor_copy(w2row[:, 128 + L:128 + 2 * L], w2row[:, 128:128 + L])
                dbg_dump(f"{tag}_w2row", w2row[:], (1, W2))
                _, w2d_dram, w2rep = layer_scratch(tag)
                i1 = nc.sync.dma_start(out=rap(w2d_dram, 0, [[1, 1], [1, W2]]), in_=w2row[:])
                i2 = nc.sync.dma_start(out=rap(w2rep, 0, [[PITCH + 1, 128], [1, W2]]),
                                       in_=rap(w2d_dram, 0, [[0, 128], [1, W2]]))
                add_dep_helper(i2.ins, i1.ins, True, "w2d raw")
                i3 = nc.sync.dma_start(
                    out=circ[:],
                    in_=rap(w2rep, 128 + L - 128 * (nt - 1),
                            [[PITCH, 128], [1, (2 * nt - 1) * 128]]))
                add_dep_helper(i3.ins, i2.ins, True, "w2rep raw")
                dbg_dump(f"{tag}_circ", circ[:], (128, (2 * nt - 1) * 128))
            return circ

        def roll_add(circ, vo_sb, x_sb, L, out_sb):
            tls = tiles_of(L)
            nt = len(tls)
            for ti, (t0, tl) in enumerate(tls):
                ps = pa([128, 512])
                for i, (s0, sl) in enumerate(tls):
                    mi = (ti - i) + (nt - 1)
                    nc.tensor.matmul(ps[0:tl, :],
                                     circ[0:sl, 128 * mi:128 * mi + tl],
                                     vo_sb[0:sl, 512 * i:512 * (i + 1)],
                                     start=(i == 0), stop=(i == nt - 1))
                nc.vector.tensor_tensor(out_sb[0:tl, 512 * ti:512 * (ti + 1)],
                                        x_sb[0:tl, 512 * ti:512 * (ti + 1)],
                                        ps[0:tl, :], op=ALU.add)

        def load_iat(pool, name, L, tag):
            nt = len(tiles_of(L))
            h = inp(name, (nt * 128, nt * 128))
            tiles, idx = {}, 0
            buf = pool.tile([128, (3 * nt - 2) * 128], F32, tag=tag)
            for k in range(nt):
                for i in (k - 1, k, k + 1):
                    if 0 <= i < nt and (i, k) not in tiles:
                        sl_ap = buf[:, 128 * idx:128 * (idx + 1)]
                        nc.sync.dma_start(out=sl_ap,
                                          in_=h[128 * i:128 * (i + 1), 128 * k:128 * (k + 1)])
                        tiles[(i, k)] = sl_ap
                        idx += 1
            return tiles

        def banded_seasonal(iat_tiles, x_sb, L, out_sb):
            tls = tiles_of(L)
            for k, (t0, tl) in enumerate(tls):
                ps = pa([128, 512])
                ks = [i for i in (k - 1, k, k + 1) if 0 <= i < len(tls)]
                for j, i in enumerate(ks):
                    s0, sl = tls[i]
                    nc.tensor.matmul(ps[0:tl, :],
                                     iat_tiles[(i, k)][0:sl, 0:tl],
                                     x_sb[0:sl, 512 * i:512 * (i + 1)],
                                     start=(j == 0), stop=(j == len(ks) - 1))
                nc.vector.tensor_copy(out_sb[0:tl, 512 * k:512 * (k + 1)], ps[0:tl, :])

        def ffn_block(pool, x_sb, L, TP, c1name, c2name, tag):
            """returns out_sb = x + gelu(x@c1.T)@c2.T (from pool)."""
            nt = len(tiles_of(L))
            out_sb = pool.tile([128, nt * 512], F32, tag="xbig", name=f"ffnout_{tag}")
            with tc.tile_pool(name=f"ffn_{tag}", bufs=1) as fp:
                xT = transpose_tiles(fp, x_sb, L, TP, f"ffnxT_{tag}")
                c1 = fp.tile([128, 4, DFF], F32, tag="c1w")
                nc.sync.dma_start(out=c1[:],
                                  in_=inp(c1name, (D, DFF))[:].rearrange("(a p) f -> p a f", p=128))
                c2 = fp.tile([128, 16, 512], BF16, tag="c2w")
                nc.sync.dma_start(out=c2[:],
                                  in_=inp(c2name, (DFF, D), BF16)[:].rearrange("(a p) f -> p a f", p=128))
                groups = [list(range(g, min(g + 4, nt))) for g in range(0, nt, 4)]
                for grp in groups:
                    g0, gl = grp[0] * 128, sum(tiles_of(L)[i][1] for i in grp)
                    hT = fp.tile([128, 16, 512], BF16, tag="hT")
                    for fc in range(16):
                        for (n0, nl) in [(a, min(512, gl - a)) for a in range(0, gl, 512)]:
                            ps = pa([128, 512])
                            for cc in range(4):
                                nc.tensor.matmul(ps[:, 0:nl],
                                                 c1[:, cc, 128 * fc:128 * (fc + 1)],
                                                 xT[:, cc, g0 + n0:g0 + n0 + nl],
                                                 start=(cc == 0), stop=(cc == 3))
                            nc.scalar.activation(hT[:, fc, n0:n0 + nl], ps[:, 0:nl],
                                                 AF.Gelu_apprx_tanh)
                    for ti in grp:
                        t0, tl = tiles_of(L)[ti]
                        ps = pa([128, 512])
                        for fc in range(16):
                            nc.tensor.matmul(ps[0:tl, :],
                                             hT[:, fc, t0 - g0:t0 - g0 + tl],
                                             c2[:, fc, :],
                                             start=(fc == 0), stop=(fc == 15))
                        nc.vector.tensor_tensor(out_sb[0:tl, 512 * ti:512 * (ti + 1)],
                                                x_sb[0:tl, 512 * ti:512 * (ti + 1)],
                                                ps[0:tl, :], op=ALU.add)
            return out_sb

        def my_layernorm(pool, x_sb, L, tag):
            nt = len(tiles_of(L))
            out_sb = pool.tile([128, nt * 512], F32, tag=f"mln_{tag}")
            mrow_ps = pr([1, 512])
            with tc.tile_pool(name=f"ln_{tag}", bufs=2) as lp:
                for ti, (t0, tl) in enumerate(tiles_of(L)):
                    xt = x_sb[0:tl, 512 * ti:512 * (ti + 1)]
                    mean = lp.tile([128, 1], F32, tag="lnmean")
                    nc.vector.tensor_reduce(mean[0:tl, :], xt, op=ALU.add, axis=AXX)
                    nc.vector.tensor_scalar(mean[0:tl, :], mean[0:tl, :],
                                            scalar1=1.0 / 512.0, scalar2=None, op0=ALU.mult)
                    xc = lp.tile([128, 512], F32, tag="lnxc")
                    nc.vector.tensor_scalar(xc[0:tl, :], xt, scalar1=mean[0:tl, :],
                                            scalar2=None, op0=ALU.subtract)
                    ssq = lp.tile([128, 1], F32, tag="lnssq")
                    sqt = lp.tile([128, 512], F32, tag="lnsq")
                    nc.scalar.activation(sqt[0:tl, :], xc[0:tl, :], AF.Square,
                                         accum_out=ssq[0:tl, :])
                    rstd = lp.tile([128, 1], F32, tag="lnrstd")
                    nc.scalar.activation(rstd[0:tl, :], ssq[0:tl, :], AF.Sqrt,
                                         bias=eps_col[0:tl, :], scale=1.0)
                    nc.vector.reciprocal(rstd[0:tl, :], rstd[0:tl, :])
                    xh = out_sb[0:tl, 512 * ti:512 * (ti + 1)]
                    nc.vector.tensor_scalar(xh, xc[0:tl, :], scalar1=rstd[0:tl, :],
                                            scalar2=math.sqrt(512.0), op0=ALU.mult,
                                            op1=ALU.mult)
                    nc.tensor.matmul(mrow_ps[:], ones_col[0:tl, :], xh,
                                     start=(ti == 0), stop=(ti == nt - 1))
                negm_row = lp.tile([1, 512], F32, tag="lnnegm")
                nc.vector.tensor_scalar(negm_row[:], mrow_ps[:], scalar1=-1.0 / L,
                                        scalar2=None, op0=ALU.mult)
                for ti, (t0, tl) in enumerate(tiles_of(L)):
                    bps = pa([128, 512])
                    nc.tensor.matmul(bps[0:tl, :], ones_row[:, 0:tl], negm_row[:],
                                     start=True, stop=True)
                    nc.vector.tensor_tensor(out_sb[0:tl, 512 * ti:512 * (ti + 1)],
                                            out_sb[0:tl, 512 * ti:512 * (ti + 1)],
                                            bps[0:tl, :], op=ALU.add)
            return out_sb

        def ac_layer(pool, x_q_sb, xT_q, kvT_src, L, topk, wtag, out_sb):
            TP = 1024 if L > 640 else 640
            with tc.tile_pool(name=f"acw_{wtag}", bufs=1) as awp:
                wq_sb = awp.tile([128, 4, 512], F32, tag="wq")
                nc.sync.dma_start(out=wq_sb[:],
                                  in_=inp(f"{wtag}_wqT", (D, D))[:].rearrange("(a p) f -> p a f", p=128))
                wk_sb = awp.tile([128, 4, 512], F32, tag="wk")
                nc.sync.dma_start(out=wk_sb[:],
                                  in_=inp(f"{wtag}_wkT", (D, D))[:].rearrange("(a p) f -> p a f", p=128))
                wvo_sb = awp.tile([128, 4, 512], F32, tag="wvo")
                nc.sync.dma_start(out=wvo_sb[:],
                                  in_=inp(f"{wtag}_wvoT", (D, D))[:].rearrange("(a p) f -> p a f", p=128))
                bq_sb = awp.tile([128, 4], F32, tag="bq")
                nc.sync.dma_start(out=bq_sb[:], in_=inp(f"{wtag}_bq", (128, 4))[:])
                bk_sb = awp.tile([128, 4], F32, tag="bk")
                nc.sync.dma_start(out=bk_sb[:], in_=inp(f"{wtag}_bk", (128, 4))[:])
                bvo_sb = awp.tile([1, D], F32, tag="bvo")
                nc.sync.dma_start(out=bvo_sb[:], in_=inp(f"{wtag}_bvo", (1, D))[:])

                with tc.tile_pool(name=f"accirc_{wtag}", bufs=1) as circp:
                    with tc.tile_pool(name=f"acqk_{wtag}", bufs=1) as qkp:
                        kT = projT(qkp, wk_sb, bk_sb, kvT_src, L, TP, "kT")
                        qT = projT(qkp, wq_sb, bq_sb, xT_q, L, TP, "qT")
                        crow_ps = gram_corr(qT, kT, L, wtag)
                    circ = filter_from_corr(circp, crow_ps, L, topk, wtag)
                    with tc.tile_pool(name=f"acvo_{wtag}", bufs=1) as vop:
                        vo_sb = proj_normal(vop, kvT_src, wvo_sb, bvo_sb, L, "vo")
                        roll_add(circ, vo_sb, x_q_sb, L, out_sb)

        # =========================================================
        # Phase 0: LIN + inits + embeddings
        # =========================================================
        xn = pp.tile([128, 8, OUT_V], F32, tag="xn")
        mean_row = pp.tile([1, OUT_V], F32, tag="meanrow")
        xd_sb = pp.tile([128, 5 * 512], F32, tag="x_dec_stream")

        with tc.tile_pool(name="ph0", bufs=1) as p0:
            xr = p0.tile([128, 8, OUT_V], F32, tag="xraw")
            nc.sync.dma_start(out=xr[:], in_=x_in[:].rearrange("(a p) c -> p a c", p=128))
            sum_ps = pr([1, OUT_V])
            for a in range(8):
                nc.tensor.matmul(sum_ps[:], ones_col[:], xr[:, a, :],
                                 start=(a == 0), stop=(a == 7))
            mu = p0.tile([1, OUT_V], F32, tag="mu")
            nc.vector.tensor_scalar(mu[:], sum_ps[:], scalar1=1.0 / 1024.0,
                                    scalar2=None, op0=ALU.mult)
            sq = p0.tile([128, 8, OUT_V], F32, tag="xsq")
            nc.scalar.activation(sq[:], xr[:], AF.Square)
            ssq_ps = pr([1, OUT_V])
            for a in range(8):
                nc.tensor.matmul(ssq_ps[:], ones_col[:], sq[:, a, :],
                                 start=(a == 0), stop=(a == 7))
            var = p0.tile([1, OUT_V], F32, tag="var")
            nc.vector.tensor_scalar(var[:], ssq_ps[:], scalar1=1.0 / 1024.0,
                                    scalar2=None, op0=ALU.mult)
            musq = p0.tile([1, OUT_V], F32, tag="musq")
            nc.scalar.activation(musq[:], mu[:], AF.Square)
            nc.vector.tensor_tensor(var[:], var[:], musq[:], op=ALU.subtract)
            std = p0.tile([1, OUT_V], F32, tag="std")
            nc.scalar.activation(std[:], var[:], AF.Sqrt)
            rstd = p0.tile([1, OUT_V], F32, tag="rstd")
            nc.vector.reciprocal(rstd[:], std[:])
            negmu = p0.tile([1, OUT_V], F32, tag="negmu")
            nc.vector.tensor_scalar(negmu[:], mu[:], scalar1=-1.0, scalar2=None, op0=ALU.mult)
            bmu_ps = pa([128, OUT_V])
            nc.tensor.matmul(bmu_ps[:], ones_row[:], negmu[:], start=True, stop=True)
            bmu = p0.tile([128, OUT_V], F32, tag="bmusb")
            nc.vector.tensor_copy(bmu[:], bmu_ps[:])
            brs_ps = pa([128, OUT_V])
            nc.tensor.matmul(brs_ps[:], ones_row[:], rstd[:], start=True, stop=True)
            brs = p0.tile([128, OUT_V], F32, tag="brssb")
            nc.vector.tensor_copy(brs[:], brs_ps[:])
            for a in range(8):
                nc.vector.tensor_tensor(xn[:, a, :], xr[:, a, :], bmu[:], op=ALU.add)
                nc.vector.tensor_tensor(xn[:, a, :], xn[:, a, :], brs[:], op=ALU.mult)
            gt_row = p0.tile([1, OUT_V], F32, tag="gtrow")
            nc.sync.dma_start(out=gt_row[:], in_=xn[127:128, 7, :])
            nc.sync.dma_start(out=out_gt[:], in_=gt_row[:])

            xm_ps = pr([1, OUT_V])
            for a in range(8):
                pl = 128 if a < 7 else 127
                nc.tensor.matmul(xm_ps[:], ones_col[0:pl, :], xn[0:pl, a, :],
                                 start=(a == 0), stop=(a == 7))
            nc.vector.tensor_scalar(mean_row[:], xm_ps[:], scalar1=1.0 / 1023.0,
                                    scalar2=None, op0=ALU.mult)

            xinT = p0.tile([21, 1026], F32, tag="xinT")
            for a in range(8):
                pl = 128 if a < 7 else 127
                tp_ = pa([21, 128])
                nc.tensor.transpose(tp_[:, 0:pl], xn[0:pl, a, :], ident[0:pl, 0:pl])
                nc.vector.tensor_copy(xinT[:, 1 + 128 * a:1 + 128 * a + pl], tp_[:, 0:pl])
            nc.vector.tensor_copy(xinT[:, 0:1], xinT[:, 1023:1024])
            nc.vector.tensor_copy(xinT[:, 1024:1025], xinT[:, 1:2])

            sb_bt = inp("sinit_bt", (8 * 128, 512))
            sinit = p0.tile([128, 5, OUT_V], F32, tag="sinit")
            with tc.tile_pool(name="sinitl", bufs=2) as slp:
                for k in range(4):
                    ps = pa([128, OUT_V])
                    lo = (511 + 128 * k - 12) // 128
                    hi = min(7, (511 + 128 * k + 127 + 12) // 128)
                    rng = list(range(lo, hi + 1))
                    for j, i in enumerate(rng):
                        sl = 128 if i < 7 else 127
                        bt_t = slp.tile([128, 128], F32, tag="btt")
                        nc.sync.dma_start(out=bt_t[:],
                                          in_=sb_bt[128 * i:128 * (i + 1), 128 * k:128 * (k + 1)])
                        nc.tensor.matmul(ps[:], bt_t[0:sl, :], xn[0:sl, i, :],
                                         start=(j == 0), stop=(j == len(rng) - 1))
                    nc.vector.tensor_copy(sinit[:, k, :], ps[:])
            nc.vector.memset(sinit[:, 4, :], 0.0)

            sinitT = p0.tile([21, 516], F32, tag="sinitT")
            nc.vector.memset(sinitT[:], 0.0)
            for k in range(4):
                tp_ = pa([21, 128])
                nc.tensor.transpose(tp_[:], sinit[:, k, :], ident[:])
                nc.vector.tensor_copy(sinitT[:, 1 + 128 * k:1 + 128 * (k + 1)], tp_[:])
            nc.vector.tensor_copy(sinitT[:, 514:515], sinitT[:, 1:2])

            x_sb = pp.tile([128, 8 * 512], F32, tag="x_enc_stream")
            embw = []
            for i in range(3):
                t = p0.tile([OUT_V, D], F32, tag=f"embw{i}")
                nc.sync.dma_start(out=t[:], in_=inp(f"enc_emb_{i}", (OUT_V, D))[:])
                embw.append(t)
            for ti, (t0, tl) in enumerate(tiles_of(LEN)):
                ps = pa([128, 512])
                for d_i in range(3):
                    nc.tensor.matmul(ps[0:tl, :], xinT[:, t0 + d_i:t0 + d_i + tl],
                                     embw[d_i][:], start=(d_i == 0), stop=(d_i == 2))
                nc.vector.tensor_copy(x_sb[0:tl, 512 * ti:512 * (ti + 1)], ps[0:tl, :])

            dembw = []
            for i in range(3):
                t = p0.tile([OUT_V, D], F32, tag=f"dembw{i}")
                nc.sync.dma_start(out=t[:], in_=inp(f"dec_emb_{i}", (OUT_V, D))[:])
                dembw.append(t)
            pe_in = inp("pos_emb", (LDEC, D))
            with tc.tile_pool(name="peld", bufs=2) as pel:
                for ti, (t0, tl) in enumerate(tiles_of(LDEC)):
                    ps = pa([128, 512])
                    for d_i in range(3):
                        nc.tensor.matmul(ps[0:tl, :], sinitT[:, t0 + d_i:t0 + d_i + tl],
                                         dembw[d_i][:], start=(d_i == 0), stop=(d_i == 2))
                    pet = pel.tile([128, 512], F32, tag="pet")
                    nc.sync.dma_start(out=pet[0:tl, :], in_=pe_in[t0:t0 + tl, :])
                    nc.vector.tensor_tensor(xd_sb[0:tl, 512 * ti:512 * (ti + 1)],
                                            ps[0:tl, :], pet[0:tl, :], op=ALU.add)

        dbg_dump("emb", x_sb[:], (128, 8 * 512))
        dbg_dump("demb", xd_sb[:], (128, 5 * 512))

        # =========================================================
        # Encoder
        # =========================================================
        with tc.tile_pool(name="enc_scope", bufs=1) as esc:
            iat_e = load_iat(esc, "iat_e", LEN, "iat_e")
            for li in range(2):
                with tc.tile_pool(name=f"encl{li}", bufs=1) as lp:
                    x2 = lp.tile([128, 8 * 512], F32, tag="xbig")
                    with tc.tile_pool(name=f"encT{li}", bufs=1) as tpp_:
                        xT = transpose_tiles(tpp_, x_sb, LEN, 1024, "xTe")
                        ac_layer(lp, x_sb, xT, xT, LEN, 20, f"enc{li}", x2)
                    dbg_dump(f"acout{li}", x2[:], (128, 8 * 512))
                    x2s = lp.tile([128, 8 * 512], F32, tag="x2s")
                    banded_seasonal(iat_e, x2, LEN, x2s)
                    x3 = ffn_block(lp, x2s, LEN, 1024, f"enc{li}_conv1T", f"enc{li}_conv2R", "e")
                    banded_seasonal(iat_e, x3, LEN, x_sb)
                dbg_dump(f"encl{li}", x_sb[:], (128, 8 * 512))
        enc_out = my_layernorm(pp, x_sb, LEN, "encln")
        dbg_dump("encout", enc_out[:], (128, 8 * 512))

        # =========================================================
        # Decoder
        # =========================================================
        with tc.tile_pool(name="dec_scope", bufs=1) as dsc:
            iat_d = load_iat(dsc, "iat_d", LDEC, "iat_d")
            encT = transpose_tiles(dsc, enc_out, LDEC, 640, "encT")
            tsum = dsc.tile([3, 512], F32, tag="tsum")
            arows_h = inp("a_rows_d", (5 * 128, 3))

            def trend_rows(x_cur, first):
                trow_ps = pr([3, 512])
                with tc.tile_pool(name="trl", bufs=2) as trl:
                    tls = tiles_of(LDEC)
                    for j, (s0, sl) in enumerate(tls):
                        at = trl.tile([128, 3], F32, tag="att")
                        nc.sync.dma_start(out=at[:], in_=arows_h[128 * j:128 * (j + 1), :])
                        nc.tensor.matmul(trow_ps[:], at[0:sl, :],
                                         x_cur[0:sl, 512 * j:512 * (j + 1)],
                                         start=(j == 0), stop=(j == len(tls) - 1))
                if first:
                    nc.vector.tensor_copy(tsum[:], trow_ps[:])
                else:
                    nc.vector.tensor_tensor(tsum[:], tsum[:], trow_ps[:], op=ALU.add)

            xd2 = dsc.tile([128, 5 * 512], F32, tag="xdbig")
            with tc.tile_pool(name="decT1", bufs=1) as dt1:
                xdT = transpose_tiles(dt1, xd_sb, LDEC, 640, "xdT")
                ac_layer(dsc, xd_sb, xdT, xdT, LDEC, 18, "dec_sa", xd2)
            xd2s = dsc.tile([128, 5 * 512], F32, tag="xd2s")
            banded_seasonal(iat_d, xd2, LDEC, xd2s)
            trend_rows(xd2, True)
            dbg_dump("xd2s", xd2s[:], (128, 5 * 512))

            xd3 = dsc.tile([128, 5 * 512], F32, tag="xdbig", name="xd3")
            with tc.tile_pool(name="decT2", bufs=1) as dt2:
                xd2sT = transpose_tiles(dt2, xd2s, LDEC, 640, "xd2sT")
                ac_layer(dsc, xd2s, xd2sT, encT, LDEC, 18, "dec_ca", xd3)
            xd3s = dsc.tile([128, 5 * 512], F32, tag="xd3s")
            banded_seasonal(iat_d, xd3, LDEC, xd3s)
            trend_rows(xd3, False)
            dbg_dump("xd3s", xd3s[:], (128, 5 * 512))

            xd4 = ffn_block(dsc, xd3s, LDEC, 640, "dec_conv1T", "dec_conv2R", "d")
            xd4s = dsc.tile([128, 5 * 512], F32, tag="xd4s")
            banded_seasonal(iat_d, xd4, LDEC, xd4s)
            trend_rows(xd4, False)

            dec_ln = my_layernorm(dsc, xd4s, LDEC, "decln")
            dbg_dump("decln", dec_ln[:], (128, 5 * 512))

            fin_row = dsc.tile([1, 512], F32, tag="finrow")
            nc.vector.tensor_copy(fin_row[:], dec_ln[0:1, 4 * 512:5 * 512])
            nc.sync.dma_start(out=rap(rowbuf_dram, 0, [[1, 1], [1, 512]]), in_=fin_row[:])
            nc.sync.dma_start(out=rap(rowbuf_dram, 512, [[512, 3], [1, 512]]), in_=tsum[:])
            fcol = dsc.tile([128, 4], F32, tag="fcol")
            nc.sync.dma_start(out=fcol[:], in_=rap(rowbuf_dram, 0, [[1, 128], [128, 4]]))
            tcol = dsc.tile([128, 12], F32, tag="tcol")
            nc.sync.dma_start(out=tcol[:], in_=rap(rowbuf_dram, 512, [[1, 128], [128, 12]]))

            with tc.tile_pool(name="head", bufs=1) as hp:
                pw = hp.tile([128, 4, OUT_V], F32, tag="pw")
                nc.sync.dma_start(out=pw[:],
                                  in_=inp("proj_wT", (D, OUT_V))[:].rearrange("(a p) f -> p a f", p=128))
                head_ps = pr([1, OUT_V])
                for cc in range(4):
                    nc.tensor.matmul(head_ps[:], fcol[:, cc:cc + 1], pw[:, cc, :],
                                     start=(cc == 0), stop=False)
                tw = []
                for i in range(3):
                    t = hp.tile([128, 4, OUT_V], F32, tag=f"tw{i}")
                    nc.sync.dma_start(out=t[:],
                                      in_=inp(f"trend_w{i}", (D, OUT_V))[:].rearrange("(a p) f -> p a f", p=128))
                    tw.append(t)
                pairs = [(1, 0), (2, 1), (0, 2)]
                for pi, (j, i) in enumerate(pairs):
                    for cc in range(4):
                        nc.tensor.matmul(head_ps[:],
                                         tcol[:, 4 * j + cc:4 * j + cc + 1],
                                         tw[i][:, cc, :],
                                         start=False, stop=(pi == 2 and cc == 3))
                pb = hp.tile([1, OUT_V], F32, tag="pb")
                nc.sync.dma_start(out=pb[:], in_=inp("proj_b", (1, OUT_V))[:])
                fin = hp.tile([1, OUT_V], F32, tag="fin")
                nc.vector.tensor_tensor(fin[:], head_ps[:], pb[:], op=ALU.add)
                nc.vector.tensor_tensor(fin[:], fin[:], mean_row[:], op=ALU.add)
                nc.sync.dma_start(out=out_pred[:], in_=fin[:])

    nc.compile()
    return nc, inputs, dbg_names


_CACHE = {}


def kernel(x_enc, drop, params, dbg=(), _want_results=False, trace=False):
    x_enc = np.asarray(x_enc, np.float32)
    key = tuple(sorted(dbg))
    if key not in _CACHE:
        _CACHE[key] = build_program(dbg=dbg)
    nc, inputs, dbg_names = _CACHE[key]
    consts = host_consts(params)
    in_maps = []
    for b in range(BATCH):
        m = {"x_enc": np.ascontiguousarray(x_enc[b])}
        for name in inputs:
            if name != "x_enc":
                m[name] = consts[name]
        in_maps.append(m)
    res = run_bass_kernel_spmd(nc, in_maps, core_ids=list(range(BATCH)), trace=trace)
    pred = np.stack([r["out_pred"] for r in res.results])
    gt = np.stack([r["out_gt"] for r in res.results])
    if _want_results:
        return (pred, gt), res
    return pred, gt
